# revision 17
# baseline (speedup 1.0000x reference)
"""NER head (Linear+ReLU emissions -> CRF mean NLL) on 8 NeuronCores.

Strategy: pure data-parallel over batch (8 seqs/core). The device kernel
computes emissions.T = relu(W @ emb.T + b) per core. Host pre-transposes
each slice to [H, tokens] and casts to fp8e4m3: 4x less HBM traffic than
fp32 (3.15 MB/core) and TensorE runs DoubleRow matmuls (two fp8 weights
per PE cell -> K=256 per pass, 24 matmuls instead of 96 fp32-emulated
ones). PSUM accumulates in fp32, so the only precision loss is the input
rounding (validated 2.7e-4 rel err on the final NLL vs 2e-2 tolerance).
ReLU+bias is split across ScalarE (even chunks) and VectorE (odd chunks)
to halve the activation tail; emissions stream back as fp8e5 (1.2e-4). The tiny
CRF dynamic program (~0.005% of total FLOPs) runs vectorized on host in
float64.
"""

import numpy as np
from contextlib import ExitStack

import ml_dtypes

import concourse.bass as bass  # noqa: F401  (registers bass types)
import concourse.tile as tile
from concourse import bacc, mybir
from concourse.bass_utils import run_bass_kernel_spmd

B, S, H, T = 64, 512, 768, 9
NCORES = 8
BC = B // NCORES            # sequences per core
TOK = BC * S                # tokens per core
HT = H // 128               # contraction tiles
SB = HT // 2                # DoubleRow super-blocks (K=256 each)
NT = TOK // 512             # psum free-dim chunks
TP = 16                     # T padded to 16: DoubleRow LDWEIGHTS requires the
                            # k-group stride to be a multiple of 16 bytes

FP8 = ml_dtypes.float8_e4m3

TRACE = False
LAST_RESULTS = None

_nc_cache = None


def _build_bass():
    nc = bacc.Bacc(
        "TRN2",
        target_bir_lowering=False,
        debug=False,
        enable_asserts=False,
        num_devices=NCORES,
    )
    # embT packed host-side as [128, HT*TOK] fp8: block ht at cols
    # [ht*TOK, (ht+1)*TOK) holds rows [ht*128, (ht+1)*128) of emb.T.
    embT = nc.dram_tensor(
        "embT", [128, HT * TOK], mybir.dt.float8e4, kind="ExternalInput"
    ).ap()
    # W.T packed the same way: [128, HT*T] fp8.
    wt = nc.dram_tensor("wt", [128, HT * TP], mybir.dt.float8e4, kind="ExternalInput").ap()
    bb = nc.dram_tensor("bb", [T, 1], mybir.dt.float32, kind="ExternalInput").ap()
    emT = nc.dram_tensor("emT", [T, TOK], mybir.dt.float8e5, kind="ExternalOutput").ap()

    with tile.TileContext(nc) as tc:
        with ExitStack() as ctx:
            consts = ctx.enter_context(tc.tile_pool(name="consts", bufs=1))
            inp = ctx.enter_context(tc.tile_pool(name="inp", bufs=1))
            pp = ctx.enter_context(tc.tile_pool(name="pp", bufs=1, space="PSUM"))
            op = ctx.enter_context(tc.tile_pool(name="op", bufs=1))

            # Embedding stream first on the Sync HWDGE ring so the first
            # byte lands as early as possible; weights/bias ride the
            # Scalar HWDGE ring in parallel (SDMA round-robins packets).
            # The host packs each DoubleRow super-block as two contiguous
            # token-halves ([h][k-group j][t] layout), so each 0.5MB half
            # arrives via a plain 2D DMA with 4KB-per-partition descriptors
            # (2KB strided descriptors measured ~15% slower) and matmuls
            # chase at half-super granularity.
            st_tiles = []
            for sb in range(SB):
                st = inp.tile(
                    [128, 2 * TOK], mybir.dt.float8e4,
                    name=f"st{sb}", tag=f"st{sb}", bufs=1,
                )
                if sb < SB - 1:
                    # 1MB full-super DMA: 8KB descriptors stream ~20% faster
                    nc.sync.dma_start(
                        st[:, :], embT[:, sb * 2 * TOK:(sb + 1) * 2 * TOK]
                    )
                else:
                    # last super in two 0.5MB halves so the final matmul
                    # batch starts half a super earlier
                    for h in range(2):
                        nc.sync.dma_start(
                            st[:, h * TOK:(h + 1) * TOK],
                            embT[:, sb * 2 * TOK + h * TOK:
                                 sb * 2 * TOK + (h + 1) * TOK]
                        )
                st_tiles.append(st)

            w_t = consts.tile([128, HT * TP], mybir.dt.float8e4)
            nc.scalar.dma_start(w_t[:, :], wt[:, :])
            b_t = consts.tile([T, 1], mybir.dt.float32)
            nc.scalar.dma_start(b_t[:], bb[:, :])

            psum_ts = [
                pp.tile([TP, 512], mybir.dt.float32, name=f"ps{nt}", tag=f"ps{nt}")
                for nt in range(NT)
            ]

            # Warm-up: ~3.4us of dummy matmuls on a zeroed scratch flips the
            # PE HAM clock gate to 8/8 (2.4 GHz) before the first data tile
            # lands, so no data matmul runs at the cold 1.2 GHz rate. They
            # write psum bank 0, which the real group's start=True clears.
            zscr = op.tile([128, 512], mybir.dt.float8e4)
            nc.vector.memset(zscr[:, :], 0)
            for _ in range(8):
                nc.tensor.matmul(
                    psum_ts[0][:, :],
                    zscr[:, 0:TP],
                    zscr[:, :],
                    start=True,
                    stop=True,
                )

            for sb in range(SB):
                # [128, h, 2, TOK/2]: dim2 = the two k-blocks of this super
                rhs4 = st_tiles[sb].rearrange(
                    "p (h two t) -> p h two t", h=2, two=2
                )
                lhs3 = w_t[:, sb * 2 * TP:(sb + 1) * 2 * TP].rearrange(
                    "p (two m) -> p two m", two=2
                )
                for nt in range(NT):
                    h, ts = nt // (NT // 2), (nt % (NT // 2)) * 512
                    nc.tensor.matmul(
                        psum_ts[nt][:, :],
                        lhs3,
                        rhs4[:, h, :, ts:ts + 512],
                        start=(sb == 0),
                        stop=(sb == SB - 1),
                        perf_mode=mybir.MatmulPerfMode.DoubleRow,
                    )

            # ReLU+bias spread over ScalarE / VectorE (GpSimd takes one
            # early, non-critical chunk); the last four chunks alternate
            # S/V so the post-matmul chase is ~2 chunks per engine. The
            # output streams out in three chunks chasing the activations.
            em_sb = op.tile([T, TOK], mybir.dt.float8e5)
            eng = ["s", "v", "s", "v", "s", "v", "s", "v"]
            for nt in range(NT):
                sl = slice(nt * 512, (nt + 1) * 512)
                if eng[nt] == "s":
                    nc.scalar.activation(
                        em_sb[:, sl],
                        psum_ts[nt][0:T, :],
                        mybir.ActivationFunctionType.Relu,
                        bias=b_t[:, :],
                        scale=1.0,
                    )
                else:
                    nc.vector.tensor_scalar(
                        em_sb[:, sl],
                        psum_ts[nt][0:T, :],
                        b_t[:, :],
                        0.0,
                        mybir.AluOpType.add,
                        mybir.AluOpType.max,
                    )
                if nt == 3:
                    nc.sync.dma_start(emT[:, 0:2048], em_sb[:, 0:2048])
                elif nt == 5:
                    nc.sync.dma_start(emT[:, 2048:3072], em_sb[:, 2048:3072])
            nc.sync.dma_start(emT[:, 3072:], em_sb[:, 3072:])
    nc.compile()
    return nc


def _crf_mean_nll(em, labels, mask, start_trans, transitions, end_trans):
    Bn, Sn, _ = em.shape
    valid = labels != -100
    mask_bool = (mask != 0) & valid
    labels_mod = np.where(valid, labels, 0).astype(np.int64)
    mask_f = mask_bool.astype(np.float64)
    ar = np.arange(Bn)

    first = start_trans[labels_mod[:, 0]] + em[ar, 0, labels_mod[:, 0]]
    emis_sc = np.take_along_axis(em, labels_mod[..., None], axis=2)[..., 0]
    trans_sc = transitions[labels_mod[:, :-1], labels_mod[:, 1:]]
    num = first + np.sum((emis_sc[:, 1:] + trans_sc) * mask_f[:, 1:], axis=1)
    last_idx = mask_bool.sum(axis=1).astype(np.int64) - 1
    last_lab = np.take_along_axis(labels_mod, last_idx[:, None], axis=1)[:, 0]
    num = num + end_trans[last_lab]

    alpha = start_trans[None, :] + em[:, 0]
    for s in range(1, Sn):
        x = alpha[:, :, None] + transitions[None]
        m = x.max(axis=1)
        new = m + np.log(np.exp(x - m[:, None, :]).sum(axis=1)) + em[:, s]
        alpha = np.where(mask_bool[:, s][:, None], new, alpha)
    z = alpha + end_trans[None]
    mz = z.max(axis=1)
    denom = mz + np.log(np.exp(z - mz[:, None]).sum(axis=1))
    return np.asarray((denom - num).mean(), dtype=np.float32)


def kernel(**inputs):
    global _nc_cache, LAST_RESULTS
    emb = np.asarray(inputs["embeddings"], dtype=np.float32)
    W = np.asarray(inputs["W"], dtype=np.float32)
    b = np.asarray(inputs["b"], dtype=np.float32)
    start_trans = np.asarray(inputs["start_trans"], dtype=np.float64)
    transitions = np.asarray(inputs["transitions"], dtype=np.float64)
    end_trans = np.asarray(inputs["end_trans"], dtype=np.float64)
    labels = np.asarray(inputs["labels"])
    mask = np.asarray(inputs["mask"])

    if _nc_cache is None:
        _nc_cache = _build_bass()
    nc = _nc_cache

    # [H, T] -> pad T to TP -> blocked [128, HT*TP] fp8
    WTp = np.zeros((H, TP), dtype=np.float32)
    WTp[:, :T] = W.T
    wt_np = np.ascontiguousarray(
        WTp.reshape(HT, 128, TP).transpose(1, 0, 2).reshape(128, HT * TP)
    ).astype(FP8)
    bb_np = np.ascontiguousarray(b.reshape(T, 1), dtype=np.float32)
    in_maps = []
    for c in range(NCORES):
        sl = emb[c * BC:(c + 1) * BC].reshape(TOK, H)
        # [TOK, H] -> [H, TOK] -> [128, sb, h, j, t] fp8: within each
        # super-block sb, token-half h is contiguous with the two k-group
        # blocks (j) side by side -- matches the device's [h][j][t] view.
        b3 = sl.T.reshape(HT, 128, TOK).transpose(1, 0, 2)
        b5 = b3.reshape(128, SB, 2, 2, TOK // 2)
        et = np.ascontiguousarray(
            b5.transpose(0, 1, 3, 2, 4).reshape(128, HT * TOK)
        ).astype(FP8)
        in_maps.append({"embT": et, "wt": wt_np, "bb": bb_np})

    res = run_bass_kernel_spmd(
        nc, in_maps, core_ids=list(range(NCORES)), trace=TRACE
    )
    LAST_RESULTS = res
    em = np.concatenate(
        [np.asarray(r["emT"]).astype(np.float32).T.reshape(BC, S, T)
         for r in res.results],
        axis=0,
    ).astype(np.float64)
    return _crf_mean_nll(em, labels, mask, start_trans, transitions, end_trans)
